# revision 16
# baseline (speedup 1.0000x reference)
"""GCMCGraphConv forward on 8 trn2 NeuronCores (Bass/Tile).

reference:
    rf  = review_feat @ w_review.T                      [E, F]
    msg = (x[src] + weight[src] + rf) * ci[src]         [E, F]
    h   = segment_sum(msg, dst, N)                      [N, F]
    out = h * ci

Strategy (dst-owner sharding, windowed batched gathers, bf16 matmuls):
  - Core c owns nodes [c*NPC, (c+1)*NPC).  Host routes every edge to the
    owner of its dst and groups the core's edges by (128-node dst block k,
    30000-row src window w).  Tile counts per (k,w) are maxed across cores
    so all 8 cores run one SPMD program.
  - Node table T = [x | weight] in bf16 (256B rows).  Edges gathered with
    dma_gather (int16 indices relative to the 30000-row window) batched up
    to 8 tiles (1024 rows) per instruction to amortize the ~1us SWDGE
    descriptor-generation overhead that dominated the per-tile indirect
    DMA approach.
  - Blocks are processed in panels of 8 so the per-block PSUM accumulators
    stay resident while tiles are ordered (panel, window, block) --
    a batch crosses blocks but never windows.
  - Per 128-edge tile:
      * DVE builds a ci-scaled one-hot  S[e,n] = ci[src_e]*(dstloc_e==n)
        (bf16; ci[src] ships as a host-routed per-slot metadata stream)
      * PE accumulates (bf16 in, fp32 PSUM):
          psA[n, 0:128] += S^T @ [x|w]_rows
          psB[f, n]     += rf_tile^T(stationary) @ S
  - Per block (once):  psA[:, 0:64] += B @ [w_review^T | 0]
    via matmul(lhsT=copy(psB), rhs=wrT_pad), then
    out_block = (psA[:, 0:64] + psA[:, 64:128]) * ci[dst].

Host does index math / layout / dtype casts only (routing, padding,
permutation, concatenation); all float arithmetic (gathers, messages,
sums, matmuls, scaling) runs on device.
"""

import os
import numpy as np
import ml_dtypes
from contextlib import ExitStack

import concourse.bass as bass
import concourse.tile as tile
from concourse import bacc, mybir
from concourse.bass_utils import run_bass_kernel_spmd

P = 128
F = 64
TBL_W = 2 * F        # table row: 64 x | 64 w   (bf16, 256B)
WSZ = 30000          # src window rows (int16 gather indices)
NW = 5
G = 8                # max tiles per batched gather (1024 rows = SWDGE ring)
PBLK = 8             # dst blocks per panel (PSUM residency)
CHUNK = 16           # edge-tiles per rfeat DMA chunk
RFH = 4              # rf rows per 512B DMA line (bf16)
MBATCH = 512         # tiles per metadata DMA slab
PAD_DL = 16000.0     # dstloc sentinel for pad edges -> one-hot column is 0

N_NODES = 150000
N_EDGES = 1250000
N_CORES = 8

BF16 = ml_dtypes.bfloat16

# ablation knobs (timing experiments; output wrong when set)
NOGATHER = bool(int(os.environ.get("GCMC_NOGATHER", "0")))
NOPE = bool(int(os.environ.get("GCMC_NOPE", "0")))
NOOH = bool(int(os.environ.get("GCMC_NOOH", "0")))
NORF = bool(int(os.environ.get("GCMC_NORF", "0")))
NOMETA = bool(int(os.environ.get("GCMC_NOMETA", "0")))
FAKEGATHER = bool(int(os.environ.get("GCMC_FAKEGATHER", "0")))


# --------------------------------------------------------------- host prep

def host_prep(x, weight, w_review, review_feat, ci, src, dst, n_cores):
    """Route edges to dst-owner cores, build per-core DMA-friendly arrays.

    Index math, layout and dtype casts only -- no feature arithmetic.
    """
    N, Fl = x.shape
    NPC = N // n_cores
    K = (NPC + P - 1) // P
    owner = dst // NPC
    win = src // WSZ

    # per-(core, block, window) edge lists and counts
    counts = np.zeros((n_cores, K, NW), np.int64)
    per_core = []
    for c in range(n_cores):
        sel = np.nonzero(owner == c)[0]
        blk = (dst[sel] - c * NPC) >> 7
        w = win[sel]
        order = np.lexsort((w, blk))        # by (blk, w)
        per_core.append((sel[order], blk[order], w[order]))
        np.add.at(counts, (c, blk, w), 1)

    tcnt = -(-counts.max(axis=0) // P)      # [K, NW] tiles per (k, w)
    # every block needs >= 1 tile so its PSUM/out path runs
    zero_blocks = tcnt.sum(axis=1) == 0
    tcnt[zero_blocks, 0] = 1
    # pad NT to a CHUNK multiple by extending the last (k, w) group
    NT = int(tcnt.sum())
    pad_tiles = (-NT) % CHUNK
    tcnt[K - 1, NW - 1] += pad_tiles
    NT += pad_tiles

    # global tile order: (panel, w, k); also batches (t0, g, w)
    tile_block = np.empty(NT, np.int64)
    tile_w = np.empty(NT, np.int64)
    t0_of_kw = np.empty((K, NW), np.int64)
    t = 0
    batches = []
    for j in range(0, K, PBLK):
        for w in range(NW):
            g_start = t
            for k in range(j, min(j + PBLK, K)):
                n = int(tcnt[k, w])
                t0_of_kw[k, w] = t
                tile_block[t:t + n] = k
                tile_w[t:t + n] = w
                t += n
            # batches within this (panel, w) stretch, split at MBATCH edges
            b = g_start
            while b < t:
                e = min(b + G, t)
                mb_edge = (b // MBATCH + 1) * MBATCH
                e = min(e, mb_edge)
                batches.append((b, e - b, w))
                b = e
    assert t == NT

    # first/last tile per block (in global order): first is the block's
    # first (k, w) group with tiles; last is the last one.
    first_t = np.full(K, -1, np.int64)
    last_t = np.full(K, -1, np.int64)
    for k in range(K):
        ws = [w for w in range(NW) if tcnt[k, w] > 0]
        first_t[k] = t0_of_kw[k, ws[0]]
        last_t[k] = t0_of_kw[k, ws[-1]] + tcnt[k, ws[-1]] - 1

    table = np.concatenate([x, weight], axis=1).astype(BF16)   # [N, 128]
    wrT_pad = np.ascontiguousarray(w_review.T).astype(BF16)
    iota = np.broadcast_to(np.arange(P, dtype=np.float32), (P, P))
    iota = np.ascontiguousarray(iota).astype(BF16)

    # slot -> rfeat DRAM row permutation (RFH rows per 512B DMA line)
    slot_ids = np.arange(NT * P)
    t_of = slot_ids // P
    p_of = slot_ids % P
    tl = t_of % CHUNK
    rf_row = ((((t_of // CHUNK) * (CHUNK // RFH) + tl // RFH) * P + p_of)
              * RFH + (tl % RFH))

    rf16 = review_feat.astype(BF16)
    ci_flat = ci[:, 0].astype(np.float32)

    in_maps = []
    for c in range(n_cores):
        eids, blks, ws = per_core[c]
        cnt = counts[c]                      # [K, NW]
        base = np.zeros((K, NW), np.int64)
        base.reshape(-1)[1:] = np.cumsum(cnt.reshape(-1))[:-1]
        # slot position of each edge: tiles of its (k, w) group start at
        # t0_of_kw[k, w]; edges fill slots in order.
        pos_in_grp = np.arange(len(eids)) - base[blks, ws]
        slotpos = t0_of_kw[blks, ws] * P + pos_in_grp

        slots_idx = np.zeros(NT * P, np.int16)
        slots_dl = np.full(NT * P, PAD_DL, np.float32)
        slots_ci = np.zeros(NT * P, np.float32)
        slots_idx[slotpos] = (src[eids] - ws * WSZ).astype(np.int16)
        slots_dl[slotpos] = (dst[eids] - c * NPC - blks * P).astype(np.float32)
        slots_ci[slotpos] = ci_flat[src[eids]]

        # idx stream: per tile t, 8 int16 columns; position i within the
        # tile at [i % 16, t*8 + i//16], replicated across the 8 groups of
        # 16 partitions.  (Batched gathers read a contiguous column run.)
        idx_wrapped = slots_idx.reshape(NT * P // 16, 16).T    # [16, NT*8]
        idxs = np.ascontiguousarray(
            np.tile(idx_wrapped, (8, 1)))                      # [128, NT*8]

        rf = np.zeros((NT * P, Fl), BF16)
        rf[rf_row[slotpos]] = rf16[eids]

        nodes = c * NPC + np.arange(K * P)
        cic = np.zeros(K * P, np.float32)
        v = nodes < (c + 1) * NPC
        cic[v] = ci_flat[nodes[v]]

        in_maps.append({
            "table": table,
            "wrT": wrT_pad,
            "iota": iota,
            "idxs": idxs,
            "dls": np.ascontiguousarray(slots_dl.reshape(NT, P).T),
            "cis": np.ascontiguousarray(slots_ci.reshape(NT, P).T),
            "rfs": rf,
            "cic": np.ascontiguousarray(cic.reshape(K, P).T),
        })

    meta = dict(N=N, F=Fl, NPC=NPC, K=K, NT=NT, n_cores=n_cores,
                tile_block=tile_block.tolist(), tile_w=tile_w.tolist(),
                first_t=first_t.tolist(), last_t=last_t.tolist(),
                batches=batches)
    return in_maps, meta


# ------------------------------------------------------------- bass program

def build_program(meta, reps=1):
    """Build the SPMD program.  reps>1 wraps the whole kernel in a hardware
    loop that re-executes it (idempotently) for wall-clock timing."""
    N = meta["N"]; Fl = meta["F"]; NPC = meta["NPC"]; K = meta["K"]
    NT = meta["NT"]; n_cores = meta["n_cores"]
    tile_block = meta["tile_block"]; tile_w = meta["tile_w"]
    first_t = meta["first_t"]; last_t = meta["last_t"]
    batches = meta["batches"]
    dt = mybir.dt

    batch_at = {b[0]: b for b in batches}
    batch_q = {b[0]: i % 4 for i, b in enumerate(batches)}

    nc = bacc.Bacc("TRN2", target_bir_lowering=False, debug=False,
                   enable_asserts=False, num_devices=n_cores,
                   num_swdge_queues=4)

    table = nc.dram_tensor("table", [N, TBL_W], dt.bfloat16,
                           kind="ExternalInput").ap()
    wrT = nc.dram_tensor("wrT", [Fl, Fl], dt.bfloat16,
                         kind="ExternalInput").ap()
    iota = nc.dram_tensor("iota", [P, P], dt.bfloat16,
                          kind="ExternalInput").ap()
    idxs = nc.dram_tensor("idxs", [P, NT * 8], dt.int16,
                          kind="ExternalInput").ap()
    dls = nc.dram_tensor("dls", [P, NT], dt.float32,
                         kind="ExternalInput").ap()
    cis = nc.dram_tensor("cis", [P, NT], dt.float32,
                         kind="ExternalInput").ap()
    rfs = nc.dram_tensor("rfs", [NT * P, Fl], dt.bfloat16,
                         kind="ExternalInput").ap()
    cic = nc.dram_tensor("cic", [P, K], dt.float32, kind="ExternalInput").ap()
    out = nc.dram_tensor("out", [NPC, Fl], dt.float32,
                         kind="ExternalOutput").ap()

    rf_view = rfs.rearrange("(c j p h) f -> c p j h f",
                            j=CHUNK // RFH, p=P, h=RFH)

    with tile.TileContext(nc) as tc, ExitStack() as ctx:
        consts = ctx.enter_context(tc.tile_pool(name="consts", bufs=1))
        mpool = ctx.enter_context(tc.tile_pool(name="meta", bufs=2))
        gpool = ctx.enter_context(tc.tile_pool(name="gather", bufs=6))
        rfpool = ctx.enter_context(tc.tile_pool(name="rfeat", bufs=3))
        ohpool = ctx.enter_context(tc.tile_pool(name="onehot", bufs=12))
        opool = ctx.enter_context(tc.tile_pool(name="outs", bufs=6))
        btpool = ctx.enter_context(tc.tile_pool(name="btile", bufs=4))
        # PSUM pools: one bank (2KB/partition) per tile; 4 blocks' psA
        # (or psB) accumulators share one bank as column slices.
        psa = ctx.enter_context(tc.tile_pool(name="psa", bufs=4,
                                             space="PSUM"))
        psb = ctx.enter_context(tc.tile_pool(name="psb", bufs=4,
                                             space="PSUM"))

        iota_sb = consts.tile([P, P], dt.bfloat16, tag="iota")
        nc.sync.dma_start(out=iota_sb[:], in_=iota[:])
        wrT_sb = consts.tile([Fl, Fl], dt.bfloat16, tag="wrT")
        nc.sync.dma_start(out=wrT_sb[:], in_=wrT[:])
        cic_sb = consts.tile([P, K], dt.float32, tag="cic")
        nc.sync.dma_start(out=cic_sb[:], in_=cic[:])

        # panel start tiles: blocks [j, j+PBLK) share two psa + two psb
        # bank tiles allocated at the panel's first tile.
        panel_of = [int(k) // PBLK for k in tile_block]
        panel_start = {}
        for t in range(NT):
            panel_start.setdefault(panel_of[t], t)
        panel_start = {v: k for k, v in panel_start.items()}

        def body(iv=None):
            idxs_sb = dls_sb = cis_sb = rfc = None
            oh_prev = None
            gt = None
            gt_c0 = 0           # global tile index of chunk 0 of gt
            pa = {}             # half -> psum tile, for current panel
            pb = {}

            def psA_ap(k):
                kk = k % PBLK
                return pa[0][:, kk * Fl:(kk + 1) * Fl]

            def psB_ap(k):
                kk = k % PBLK
                return pb[kk // 4][:, (kk % 4) * P:(kk % 4 + 1) * P]

            for t in range(NT):
                k = tile_block[t]
                first = (t == first_t[k])
                last = (t == last_t[k])
                mb = t % MBATCH
                if t in panel_start:
                    j = panel_start[t] * PBLK
                    nblk = min(PBLK, K - j)
                    pa = {0: psa.tile([P, PBLK * Fl], dt.float32, tag="psa",
                                      name=f"psa{j}")}
                    pb = {0: psb.tile([Fl, 4 * P], dt.float32, tag="psb",
                                      name=f"psb{j}_0")}
                    if nblk > 4:
                        pb[1] = psb.tile([Fl, 4 * P], dt.float32, tag="psb",
                                         name=f"psb{j}_1")
                    # start=True resets the WHOLE PSUM bank, so only the
                    # first matmul touching each bank in a panel may use it.
                    started_a = False
                    started_b = {0: False, 1: False}
                if mb == 0:
                    wdt = min(MBATCH, NT - t)
                    idxs_sb = mpool.tile([P, MBATCH * 8], dt.int16,
                                         tag="idxs")
                    nc.sync.dma_start(out=idxs_sb[:, :wdt * 8],
                                      in_=idxs[:, t * 8:(t + wdt) * 8])
                    if not NOMETA or t == 0:
                        dls_sb = mpool.tile([P, MBATCH], dt.float32,
                                            tag="dls")
                        nc.sync.dma_start(out=dls_sb[:, :wdt],
                                          in_=dls[:, t:t + wdt])
                        cis_sb = mpool.tile([P, MBATCH], dt.float32,
                                            tag="cis")
                        nc.sync.dma_start(out=cis_sb[:, :wdt],
                                          in_=cis[:, t:t + wdt])
                if t % CHUNK == 0 and not (NORF and t > 0):
                    rfc = rfpool.tile([P, CHUNK * Fl], dt.bfloat16, tag="rfc")
                    nc.sync.dma_start(out=rfc[:], in_=rf_view[t // CHUNK])

                if t in batch_at:
                    _, g, w = batch_at[t]
                    bq = batch_q[t]
                    gt = gpool.tile([P, G * TBL_W], dt.bfloat16, tag="g")
                    gt_c0 = t
                    gv = gt[:].rearrange("p (c e) -> p c e", c=G, e=TBL_W)
                    if FAKEGATHER:
                        nc.gpsimd.dma_gather(
                            out_ap=gv[:, :, :],
                            in_ap=table[0:WSZ, :],
                            idxs_ap=idxs_sb[:, 0:G * 8],
                            num_idxs=G * P, num_idxs_reg=G * P,
                            elem_size=TBL_W, queue_num=bq)
                    elif NOGATHER:
                        pass
                    else:
                        nc.gpsimd.dma_gather(
                            out_ap=gv[:, :g, :],
                            in_ap=table[w * WSZ:w * WSZ + WSZ, :],
                            idxs_ap=idxs_sb[:, (mb) * 8:(mb + g) * 8],
                            num_idxs=g * P, num_idxs_reg=g * P,
                            elem_size=TBL_W, queue_num=bq)

                if NOOH:
                    if t == 0:
                        oh = ohpool.tile([P, P], dt.bfloat16, tag="oh")
                        nc.vector.tensor_scalar(
                            out=oh[:], in0=iota_sb[:],
                            scalar1=dls_sb[:, mb:mb + 1],
                            scalar2=cis_sb[:, mb:mb + 1],
                            op0=mybir.AluOpType.is_equal,
                            op1=mybir.AluOpType.mult)
                    else:
                        oh = oh_prev
                else:
                    oh = ohpool.tile([P, P], dt.bfloat16, tag="oh")
                    # S = (iota == dstloc) * ci_src
                    nc.vector.tensor_scalar(
                        out=oh[:], in0=iota_sb[:],
                        scalar1=dls_sb[:, mb:mb + 1],
                        scalar2=cis_sb[:, mb:mb + 1],
                        op0=mybir.AluOpType.is_equal, op1=mybir.AluOpType.mult)
                oh_prev = oh

                cg = t - gt_c0
                if NOPE:
                    if last:
                        o2 = opool.tile([P, Fl], dt.float32, tag="o2")
                        nc.vector.tensor_scalar_mul(o2[:], iota_sb[:, 0:Fl],
                                                    cic_sb[:, k:k + 1])
                        rows = min(P, NPC - k * P)
                        nc.sync.dma_start(out=out[k * P:k * P + rows, :],
                                          in_=o2[:rows, :])
                    continue
                nc.tensor.matmul(psA_ap(k), lhsT=oh[:],
                                 rhs=gt[:, cg * TBL_W:cg * TBL_W + Fl],
                                 start=not started_a, stop=False,
                                 skip_group_check=True)
                started_a = True
                nc.tensor.matmul(psA_ap(k), lhsT=oh[:],
                                 rhs=gt[:, cg * TBL_W + Fl:(cg + 1) * TBL_W],
                                 start=False, stop=False,
                                 skip_group_check=True)
                tl = t % CHUNK
                hb = (k % PBLK) // 4
                nc.tensor.matmul(psB_ap(k), lhsT=rfc[:, tl * Fl:(tl + 1) * Fl],
                                 rhs=oh[:], start=not started_b[hb], stop=last,
                                 skip_group_check=True)
                started_b[hb] = True

                if last:
                    pA = psA_ap(k)
                    bt = btpool.tile([Fl, P], dt.bfloat16, tag="bt")
                    nc.scalar.copy(bt[:], psB_ap(k))
                    nc.tensor.matmul(pA, lhsT=bt[:], rhs=wrT_sb[:],
                                     start=False, stop=True,
                                     skip_group_check=True)
                    o2 = opool.tile([P, Fl], dt.float32, tag="o2")
                    nc.vector.tensor_scalar_mul(o2[:], pA,
                                                cic_sb[:, k:k + 1])
                    rows = min(P, NPC - k * P)
                    nc.sync.dma_start(out=out[k * P:k * P + rows, :],
                                      in_=o2[:rows, :])

        if reps == 1:
            body()
        else:
            with tc.For_i(0, reps, 1) as iv:
                body(iv)

    nc.compile()
    return nc


# ------------------------------------------------------------------ driver

_CACHE = {}


def _get_program(meta, reps=1):
    key = (meta["N"], meta["F"], meta["NPC"], meta["K"], meta["NT"],
           meta["n_cores"], tuple(meta["tile_block"]), tuple(meta["tile_w"]),
           tuple(b[0] for b in meta["batches"]), reps)
    if key not in _CACHE:
        _CACHE[key] = build_program(meta, reps=reps)
    return _CACHE[key]


def run(inputs, n_cores=N_CORES, trace=False, reps=1):
    in_maps, meta = host_prep(
        inputs["x"], inputs["weight"], inputs["w_review"],
        inputs["review_feat"], inputs["ci"], inputs["src"], inputs["dst"],
        n_cores)
    nc = _get_program(meta, reps=reps)
    res = run_bass_kernel_spmd(nc, in_maps, list(range(n_cores)), trace=trace)
    outp = np.concatenate([res.results[c]["out"] for c in range(n_cores)],
                          axis=0)
    return outp, res


def kernel(**inputs) -> np.ndarray:
    inputs = {k: np.asarray(v) for k, v in inputs.items()}
    last = None
    for attempt in range(3):
        try:
            outp, _ = run(inputs, n_cores=N_CORES)
            return outp
        except Exception as e:          # transient accelerator errors
            last = e
    raise last
